# revision 14
# baseline (speedup 1.0000x reference)
"""Trainium2 Bass kernel for a 2-layer edge-featured GAT + mean-pool + FC.

Sharding: 256 graphs are split 32-per-core across 8 cores. Because `batch` is
sorted, each core owns a contiguous node range (graph-aligned), so both the
per-destination softmax segments and the mean-pool segments are core-local.
Edges are assigned to the core that owns their destination node. Between
layers, per-core node tables (features + src-attention logits, fp16) are
AllGathered so every core can gather arbitrary source rows.

Edge phase (per layer): edges sorted by dst are packed into 128-edge chunks
grouped by destination node tile (128 nodes). Per half-tile group of CH
chunks, a single dma_gather (prepare_only + trigger_dma on a rotating SWDGE
queue, so descriptor generation pipelines with DMA drains) pulls fp16
source-node rows [h | a_src | pad]. The dst-side a_dst term needs no gather:
it is taken from the core-local a_dst table via a transposed one-hot matmul
(ohT[d,e] @ a_dst_tile[d,:]). Attention weights are
p = exp(leaky_relu(a_src+a_dst+w*q)) computed as max(exp(x), exp(0.2x)),
messages are h*p, and the segment-sum over destinations is a one-hot matmul
accumulated in PSUM — with p itself carried as extra columns to produce the
softmax denominators. Softmax normalization happens once per node after
aggregation: out = (sum p*h) / (sum p + 1e-16), exactly equivalent to the
reference's per-edge normalization (the max-subtraction cancels in the
ratio). Pad edge slots gather row 0 (finite data) and carry dst_local = -1
so their one-hot columns are all zero and they contribute nothing.
"""

import sys

sys.path.insert(0, "/opt/trn_rl_repo")

import math
from contextlib import ExitStack

import numpy as np

import concourse.bacc as bacc
import concourse.bass as bass
import concourse.mybir as mybir
import concourse.tile as tile
from concourse.bass_utils import run_bass_kernel_spmd
from concourse.masks import make_identity

P = 128
NCORES = 8
NQ = 4  # SWDGE queues
SP = False  # dma_gather single_packet
PREP = False  # prepare_only path broken on HW

FULL_CFG = dict(N=20000, E=640000, FIN=128, HID=64, HEADS=4, NG=256, OUT=32)

F32 = mybir.dt.float32
F16 = mybir.dt.float16
I16 = mybir.dt.int16

ROW1E = 384  # fp16 elems per layer-1 table row: h(256) asrc(4) pad(124)
ROW2E = 128  # fp16 elems per layer-2 table row: h2(64) asrc2(1) pad(63)


# ---------------------------------------------------------------------------
# Host-side preparation: integer index manipulation + array reordering only.
# ---------------------------------------------------------------------------
def prepare(inputs, cfg):
    N, E, FIN, HID, HEADS, NG, OUT = (
        cfg["N"], cfg["E"], cfg["FIN"], cfg["HID"], cfg["HEADS"], cfg["NG"],
        cfg["OUT"],
    )
    GPC = NG // NCORES  # graphs per core

    x = np.asarray(inputs["x"], np.float32)
    ei = np.asarray(inputs["edge_index"], np.int64)
    ea = np.asarray(inputs["edge_attr"], np.float32)
    batch = np.asarray(inputs["batch"], np.int64)
    src, dst = ei[0], ei[1]

    # node ranges per core (graph-aligned; batch is sorted)
    bounds = np.searchsorted(batch, np.arange(NCORES + 1) * GPC)
    node_cnt = np.diff(bounds)
    NT = max(1, math.ceil(node_cnt.max() / P))
    NSLICE = NT * P
    NROWS = NCORES * NSLICE
    assert NROWS < 32768, f"int16 gather index overflow: {NROWS}"

    core_of_node = np.minimum(batch // GPC, NCORES - 1).astype(np.int64)
    rowid = np.empty(N, np.int64)
    for c in range(NCORES):
        ns, ne = bounds[c], bounds[c + 1]
        rowid[ns:ne] = c * NSLICE + np.arange(ne - ns)

    # edges sorted by dst; since batch is sorted, core blocks are contiguous
    order = np.argsort(dst, kind="stable")
    dsts = dst[order]
    srcs = src[order]
    ws = ea[order, 0]
    ecore = core_of_node[dsts]
    ebounds = np.searchsorted(ecore, np.arange(NCORES + 1))

    # chunks-per-tile: max over all (core, tile), rounded up to even
    cpt_max = 1
    tile_edge_counts = []
    for c in range(NCORES):
        es, ee = ebounds[c], ebounds[c + 1]
        dln = dsts[es:ee] - bounds[c]
        tid = dln // P
        cnts = np.bincount(tid, minlength=NT)
        tile_edge_counts.append(cnts)
        if len(cnts):
            cpt_max = max(cpt_max, math.ceil(cnts.max() / P))
    CPT = cpt_max + (cpt_max % 2)  # even
    CPT = max(CPT, 2)
    CH = CPT // 2
    NCHUNK = NT * CPT

    per_core = []
    for c in range(NCORES):
        ns, ne = bounds[c], bounds[c + 1]
        es, ee = ebounds[c], ebounds[c + 1]
        nloc = ne - ns

        xs = np.zeros((NSLICE, FIN), np.float16)
        xs[:nloc] = x[ns:ne].astype(np.float16)

        gl = np.full((NT * P,), -1.0, np.float32)
        gl[:nloc] = (batch[ns:ne] - c * GPC).astype(np.float32)
        gl_dev = gl.reshape(NT, P).T.copy()  # [128, NT]

        srcrow = np.zeros((NT, CPT * P), np.int64)
        dstl = np.full((NT, CPT * P), -1.0, np.float32)
        wv = np.zeros((NT, CPT * P), np.float32)

        dln = dsts[es:ee] - ns
        tid = dln // P
        cnts = tile_edge_counts[c]
        off = np.zeros(NT + 1, np.int64)
        off[1:NT + 1] = np.cumsum(cnts[:NT])
        for t in range(NT):
            k = int(cnts[t]) if t < len(cnts) else 0
            if k == 0:
                continue
            sel = slice(es + int(off[t]), es + int(off[t]) + k)
            srcrow[t, :k] = rowid[srcs[sel]]
            dstl[t, :k] = (dln[int(off[t]):int(off[t]) + k] % P).astype(
                np.float32)
            wv[t, :k] = ws[sel]

        # device layouts
        dstl_dev = dstl.reshape(NCHUNK, P).T.astype(np.float16).copy()
        wv_dev = wv.reshape(NCHUNK, P).T.astype(np.float16).copy()
        # dstl replicated across partitions: [128, NCHUNK*P], row-major per
        # chunk, every partition identical (for the transposed one-hot build)
        dstl_rep = np.tile(
            dstl.reshape(1, NCHUNK * P), (P, 1)).astype(np.float16)

        def wrap_idx(arr):  # [NT, CPT*P] -> [128, NT*CPT*8] int16
            blocks = []
            for t in range(NT):
                a = arr[t].reshape(CPT * 8, 16).T  # [16, CPT*8]
                blocks.append(np.tile(a, (8, 1)))
            return np.ascontiguousarray(
                np.concatenate(blocks, axis=1)).astype(np.int16)

        per_core.append(dict(
            xs=xs, gl=gl_dev, dstl=dstl_dev, dstlr=dstl_rep, wv=wv_dev,
            idxs=wrap_idx(srcrow),
        ))

    # weight-side constants (tiny, host-replicated)
    W1 = np.asarray(inputs["W1"], np.float32).astype(np.float16)  # [FIN,H*HID]
    W2 = np.asarray(inputs["W2"], np.float32).astype(np.float16)  # [H*HID,HID]
    as1 = np.asarray(inputs["att_src1"], np.float32).reshape(-1)
    ad1 = np.asarray(inputs["att_dst1"], np.float32).reshape(-1)
    as2 = np.asarray(inputs["att_src2"], np.float32).reshape(-1)
    ad2 = np.asarray(inputs["att_dst2"], np.float32).reshape(-1)
    q1 = (np.asarray(inputs["We1"], np.float32).reshape(HEADS, HID)
          * np.asarray(inputs["att_edge1"], np.float32)).sum(axis=1)  # [H]
    q2 = float((np.asarray(inputs["We2"], np.float32).reshape(-1)
                * np.asarray(inputs["att_edge2"], np.float32).reshape(-1))
               .sum())
    b1 = np.asarray(inputs["b1"], np.float32)
    b2 = np.asarray(inputs["b2"], np.float32)
    fcW = np.asarray(inputs["fcW"], np.float32)
    fcb = np.asarray(inputs["fcb"], np.float32)

    rep = lambda vv: np.tile(vv[None, :].astype(np.float32), (P, 1)).copy()
    rep16 = lambda vv: np.tile(vv[None, :].astype(np.float16), (P, 1)).copy()
    consts = dict(
        W1=W1, W2=W2,
        as1b=rep(as1), ad1b=rep(ad1), b1b=rep16(b1),
        as2b=rep(as2), ad2b=rep(ad2), b2b=rep(b2),
        q1b=rep16(q1), q2b=np.full((P, 1), q2, np.float32),
        fcw=fcW, fcbb=rep(fcb),
        iota=np.tile(np.arange(P, dtype=np.float32)[None, :], (P, 1)).copy(),
        iota16=np.tile(np.arange(P, dtype=np.float16)[None, :], (P, 1)).copy(),
        pidx32=np.arange(P, dtype=np.float32)[:, None].copy(),
    )

    in_maps = []
    for c in range(NCORES):
        m = dict(per_core[c])
        m.update(consts)
        in_maps.append(m)

    meta = dict(NT=NT, CPT=CPT, CH=CH, NSLICE=NSLICE, NROWS=NROWS,
                GPC=GPC, **cfg)
    return in_maps, meta


# ---------------------------------------------------------------------------
# Device program.
# ---------------------------------------------------------------------------
def build(meta, reps=1, num_devices=NCORES):
    NT, CPT, CH = meta["NT"], meta["CPT"], meta["CH"]
    NSLICE, NROWS, GPC = meta["NSLICE"], meta["NROWS"], meta["GPC"]
    FIN, HID, HEADS, OUT = meta["FIN"], meta["HID"], meta["HEADS"], meta["OUT"]
    D1 = HEADS * HID          # 256
    OFF1 = D1                 # asrc offset in row1
    OFF2 = HID                # asrc offset in row2
    NI = CPT * P              # idxs per gather (one per tile)
    NIc = NI // 16            # idx columns per gather
    NCHUNK = NT * CPT
    A = mybir.AluOpType
    ACT = mybir.ActivationFunctionType
    X = mybir.AxisListType.X
    rg = [list(range(NCORES))]

    nc = bacc.Bacc("TRN2", target_bir_lowering=False, debug=False,
                   num_devices=num_devices,
                   dynamic_dma_scratch_size=65536,
                   num_swdge_queues=NQ)

    dma_sems = [nc.alloc_semaphore(f"gsem{q}") for q in range(NQ)]

    def din(name, shape, dtype=F32):
        return nc.dram_tensor(name, list(shape), dtype,
                              kind="ExternalInput").ap()

    xs = din("xs", (NSLICE, FIN), F16)
    idxs_d = din("idxs", (P, NT * CPT * 8), I16)
    dstl_d = din("dstl", (P, NCHUNK), F16)
    dstlr_d = din("dstlr", (P, NCHUNK * P), F16)
    wv_d = din("wv", (P, NCHUNK), F16)
    gl_d = din("gl", (P, NT))
    W1_d = din("W1", (FIN, D1), F16)
    W2_d = din("W2", (D1, HID), F16)
    as1_d = din("as1b", (P, D1))
    ad1_d = din("ad1b", (P, D1))
    b1_d = din("b1b", (P, D1), F16)
    as2_d = din("as2b", (P, HID))
    ad2_d = din("ad2b", (P, HID))
    b2_d = din("b2b", (P, HID))
    q1_d = din("q1b", (P, HEADS), F16)
    q2_d = din("q2b", (P, 1))
    fcw_d = din("fcw", (HID, OUT))
    fcb_d = din("fcbb", (P, OUT))
    iota_d = din("iota", (P, P))
    iota16_d = din("iota16", (P, P), F16)
    pidx32_d = din("pidx32", (P, 1), F32)

    out_d = nc.dram_tensor("out", [GPC, OUT], F32, kind="ExternalOutput").ap()
    dbg = meta.get("debug")
    if dbg:
        t1dbg_d = nc.dram_tensor("t1dbg", [NSLICE, ROW1E], F16,
                                 kind="ExternalOutput").ap()
        o1dbg_d = nc.dram_tensor("o1dbg", [P, NT * D1], F16,
                                 kind="ExternalOutput").ap()
        ad1dbg_d = nc.dram_tensor("ad1dbg", [P, NT * HEADS], F16,
                                  kind="ExternalOutput").ap()
        t2dbg_d = nc.dram_tensor("t2dbg", [NSLICE, ROW2E], F16,
                                 kind="ExternalOutput").ap()
        pldbg_d = nc.dram_tensor("pldbg", [GPC, HID + 1], F32,
                                 kind="ExternalOutput").ap()

    with tile.TileContext(nc) as tc, ExitStack() as st:
        constp = st.enter_context(tc.tile_pool(name="constp", bufs=1))
        drp = st.enter_context(tc.tile_pool(name="drp", bufs=1, space="DRAM"))

        # whole-kernel constants
        iota_sb = constp.tile([P, P], F32)
        nc.sync.dma_start(iota_sb[:], iota_d[:])
        iota16_sb = constp.tile([P, P], F16)
        nc.sync.dma_start(iota16_sb[:], iota16_d[:])
        pidx_sb = constp.tile([P, 1], F32)
        nc.sync.dma_start(pidx_sb[:], pidx32_d[:])
        ident16 = constp.tile([P, P], F16)
        nc.vector.tensor_scalar(out=ident16[:], in0=iota16_sb[:],
                                scalar1=pidx_sb[:, 0:1], scalar2=None,
                                op0=A.is_equal)
        ident32 = constp.tile([P, P], F32)
        make_identity(nc, ident32[:])
        dstl_sb = constp.tile([P, NCHUNK], F16)
        nc.sync.dma_start(dstl_sb[:], dstl_d[:])
        wv_sb = constp.tile([P, NCHUNK], F16)
        nc.sync.dma_start(wv_sb[:], wv_d[:])
        gl_sb = constp.tile([P, NT], F32)
        nc.sync.dma_start(gl_sb[:], gl_d[:])
        q1_sb = constp.tile([P, HEADS], F16)
        nc.sync.dma_start(q1_sb[:], q1_d[:])
        q2_sb = constp.tile([P, 1], F32)
        nc.sync.dma_start(q2_sb[:], q2_d[:])
        ixs_all = constp.tile([P, NT * CPT * 8], I16)
        nc.sync.dma_start(ixs_all[:], idxs_d[:])
        adst1_sb = constp.tile([P, NT, HEADS], F16)
        adst2_sb = constp.tile([P, NT, 1], F16)

        qctr = [0]

        def gather(G, table, gbase, ni, elem, first=False):
            q = qctr[0] % NQ
            qctr[0] += 1
            if PREP and not first:
                nc.gpsimd.dma_gather(
                    G[:], table, ixs_all[:, gbase:gbase + NIc],
                    ni, ni, elem, single_packet=SP,
                    prepare_only=True, sem=dma_sems[q], queue_num=q)
                nc.gpsimd.trigger_dma(count=None, queue_num=q)
            else:
                nc.gpsimd.dma_gather(
                    G[:], table, ixs_all[:, gbase:gbase + NIc],
                    ni, ni, elem, single_packet=SP, queue_num=q)

        for _rep in range(reps):
            t1loc = drp.tile([NSLICE, ROW1E], F16, name=f"t1loc{_rep}")
            t1full = drp.tile([NROWS, ROW1E], F16, addr_space="Shared",
                              name=f"t1full{_rep}")
            t2loc = drp.tile([NSLICE, ROW2E], F16, name=f"t2loc{_rep}")
            t2full = drp.tile([NROWS, ROW2E], F16, addr_space="Shared",
                              name=f"t2full{_rep}")

            # ---------------- Phase 0: h1 = x @ W1, a_src/a_dst, table1 -----
            with tc.tile_pool(name="ph0", bufs=1) as sp, \
                 tc.tile_pool(name="ph0b", bufs=2) as sp2, \
                 tc.tile_pool(name="ph0p", bufs=2, space="PSUM") as pp:
                w1_sb = sp.tile([P, D1], F16)
                nc.sync.dma_start(w1_sb[:], W1_d[:])
                as1_sb = sp.tile([P, D1], F32)
                nc.sync.dma_start(as1_sb[:], as1_d[:])
                ad1_sb = sp.tile([P, D1], F32)
                nc.sync.dma_start(ad1_sb[:], ad1_d[:])
                xall = sp.tile([P, NT, FIN], F16)
                nc.sync.dma_start(xall[:],
                                  xs[:].rearrange("(t p) f -> p t f", p=P))
                for t in range(NT):
                    xT_ps = pp.tile([P, P], F16, space="PSUM")
                    nc.tensor.transpose(xT_ps[:], xall[:, t, :], ident16[:])
                    xT = sp2.tile([P, P], F16)
                    nc.vector.tensor_copy(out=xT[:], in_=xT_ps[:])
                    h_ps = pp.tile([P, D1], F32, space="PSUM")
                    nc.tensor.matmul(h_ps[:], lhsT=xT[:], rhs=w1_sb[:],
                                     start=True, stop=True)
                    t1t = sp2.tile([P, ROW1E], F16)
                    tmp = sp2.tile([P, D1], F32)
                    red = sp2.tile([P, HEADS], F32)
                    nc.vector.tensor_tensor(out=tmp[:], in0=h_ps[:],
                                            in1=as1_sb[:], op=A.mult)
                    nc.vector.tensor_reduce(
                        out=red[:],
                        in_=tmp[:].rearrange("p (h f) -> p h f", h=HEADS),
                        axis=X, op=A.add)
                    nc.vector.tensor_copy(out=t1t[:, OFF1:OFF1 + HEADS],
                                          in_=red[:])
                    nc.vector.tensor_tensor(out=tmp[:], in0=h_ps[:],
                                            in1=ad1_sb[:], op=A.mult)
                    nc.vector.tensor_reduce(
                        out=red[:],
                        in_=tmp[:].rearrange("p (h f) -> p h f", h=HEADS),
                        axis=X, op=A.add)
                    nc.vector.tensor_copy(out=adst1_sb[:, t, :], in_=red[:])
                    nc.vector.tensor_copy(out=t1t[:, 0:D1], in_=h_ps[:])
                    nc.vector.memset(t1t[:, OFF1 + HEADS:ROW1E], 0.0)
                    nc.sync.dma_start(t1loc[t * P:(t + 1) * P, :], t1t[:])
                    if dbg:
                        nc.sync.dma_start(t1dbg_d[t * P:(t + 1) * P, :],
                                          t1t[:])
                nc.gpsimd.collective_compute(
                    "AllGather", A.bypass, replica_groups=rg,
                    ins=[t1loc[:]], outs=[t1full[:]])

            # ---------------- Phase 1: layer-1 edge phase -------------------
            with tc.tile_pool(name=f"outp{_rep}", bufs=1) as outp:
              out1 = outp.tile([P, NT * D1], F16, name=f"out1_{_rep}")
              with tc.tile_pool(name="p1g", bufs=2) as pg, \
                   tc.tile_pool(name="p1r", bufs=2) as pr, \
                   tc.tile_pool(name="p1w", bufs=2) as pw, \
                   tc.tile_pool(name="p1oh", bufs=2) as poh, \
                   tc.tile_pool(name="p1ps", bufs=3, space="PSUM") as pps, \
                   tc.tile_pool(name="p1ps2", bufs=3, space="PSUM") as pps2:
                  for t in range(NT):
                      acc = pps.tile([P, D1 + HEADS], F32, space="PSUM")
                      gbase = t * NIc
                      cbase = t * CPT
                      G = pg.tile([P, CPT, ROW1E], F16)
                      gather(G, t1full[:], gbase, NI, ROW1E,
                             first=(t == 0))
                      dr = pr.tile([P, CPT, P], F16)
                      nc.sync.dma_start(
                          dr[:],
                          dstlr_d[:, cbase * P:(cbase + CPT) * P]
                          .rearrange("p (c e) -> p c e", c=CPT))
                      ohT = poh.tile([P, CPT, P], F16)
                      nc.vector.tensor_scalar(
                          out=ohT[:], in0=dr[:],
                          scalar1=pidx_sb[:, 0:1], scalar2=None,
                          op0=A.is_equal)
                      oh = poh.tile([P, CPT, P], F16)
                      nc.vector.tensor_tensor(
                          out=oh[:],
                          in0=iota16_sb[:].unsqueeze(1)
                              .to_broadcast([P, CPT, P]),
                          in1=dstl_sb[:, cbase:cbase + CPT]
                              .unsqueeze(2).to_broadcast([P, CPT, P]),
                          op=A.is_equal)
                      adst_ps = pps2.tile([P, CPT, HEADS], F32,
                                          space="PSUM")
                      for c in range(CPT):
                          nc.tensor.matmul(
                              adst_ps[:, c, :], lhsT=ohT[:, c, :],
                              rhs=adst1_sb[:, t, :],
                              start=True, stop=True,
                              skip_group_check=True)
                      asr = G[:, :, OFF1:OFF1 + HEADS]
                      adst16 = pw.tile([P, CPT, HEADS], F16)
                      nc.vector.tensor_copy(out=adst16[:], in_=adst_ps[:])
                      ae = pw.tile([P, CPT, HEADS], F16)
                      nc.vector.tensor_tensor(
                          out=ae[:],
                          in0=wv_sb[:, cbase:cbase + CPT].unsqueeze(2)
                              .to_broadcast([P, CPT, HEADS]),
                          in1=q1_sb[:].unsqueeze(1)
                              .to_broadcast([P, CPT, HEADS]),
                          op=A.mult)
                      nc.vector.tensor_tensor(out=ae[:], in0=ae[:],
                                              in1=adst16[:], op=A.add)
                      nc.vector.tensor_tensor(out=asr, in0=asr,
                                              in1=ae[:], op=A.add)
                      e2 = pw.tile([P, CPT, HEADS], F16)
                      nc.scalar.activation(out=e2[:], in_=asr,
                                           func=ACT.Exp, scale=0.2)
                      nc.scalar.activation(out=asr, in_=asr, func=ACT.Exp)
                      nc.vector.tensor_tensor(out=asr, in0=asr, in1=e2[:],
                                              op=A.max)
                      gm = G[:, :, 0:D1].rearrange(
                          "p c (h f) -> p c h f", h=HEADS)
                      nc.vector.tensor_tensor(
                          out=gm, in0=gm,
                          in1=asr.unsqueeze(3)
                              .to_broadcast([P, CPT, HEADS, HID]),
                          op=A.mult)
                      for c in range(CPT):
                          nc.tensor.matmul(
                              acc[:], lhsT=oh[:, c, :],
                              rhs=G[:, c, 0:D1 + HEADS],
                              start=(c == 0),
                              stop=(c == CPT - 1))
                      # epilogue: out1 = relu(acc/denom + b1)
                      dn = pw.tile([P, HEADS], F32)
                      nc.vector.tensor_scalar(out=dn[:],
                                              in0=acc[:, D1:D1 + HEADS],
                                              scalar1=1e-16, scalar2=None,
                                              op0=A.add)
                      rc = pw.tile([P, HEADS], F32)
                      nc.vector.reciprocal(rc[:], dn[:])
                      ob = out1[:, t * D1:(t + 1) * D1]
                      nc.vector.tensor_tensor(
                          out=ob.rearrange("p (h f) -> p h f", h=HEADS),
                          in0=acc[:, 0:D1].rearrange("p (h f) -> p h f",
                                                     h=HEADS),
                          in1=rc[:].unsqueeze(2)
                              .to_broadcast([P, HEADS, HID]),
                          op=A.mult)

              with tc.tile_pool(name="p1e", bufs=1) as pe:
                  b1_sb = pe.tile([P, D1], F16)
                  nc.sync.dma_start(b1_sb[:], b1_d[:])
                  for t in range(NT):
                      ob = out1[:, t * D1:(t + 1) * D1]
                      nc.vector.tensor_tensor(out=ob, in0=ob, in1=b1_sb[:],
                                              op=A.add)
                      nc.vector.tensor_scalar(out=ob, in0=ob, scalar1=0.0,
                                              scalar2=None, op0=A.max)
                  if dbg:
                      nc.sync.dma_start(o1dbg_d[:], out1[:])
                      nc.sync.dma_start(
                          ad1dbg_d[:],
                          adst1_sb[:].rearrange("p t h -> p (t h)"))

              # ---------------- Phase 2: h2 = relu(out1) @ W2, table2 -------
              with tc.tile_pool(name="ph2", bufs=1) as sp, \
                   tc.tile_pool(name="ph2b", bufs=2) as sp2, \
                   tc.tile_pool(name="ph2p", bufs=2, space="PSUM") as pp:
                  w2_sb = sp.tile([P, 2, HID], F16)
                  nc.sync.dma_start(w2_sb[:],
                                    W2_d[:].rearrange("(k p) n -> p k n", p=P))
                  as2_sb = sp.tile([P, HID], F32)
                  nc.sync.dma_start(as2_sb[:], as2_d[:])
                  ad2_sb = sp.tile([P, HID], F32)
                  nc.sync.dma_start(ad2_sb[:], ad2_d[:])
                  for t in range(NT):
                      h2_ps = pp.tile([P, HID], F32, space="PSUM")
                      for k in range(2):
                          hT_ps = pp.tile([P, P], F16, space="PSUM")
                          nc.tensor.transpose(
                              hT_ps[:],
                              out1[:, t * D1 + k * P:t * D1 + (k + 1) * P],
                              ident16[:])
                          hT = sp2.tile([P, P], F16)
                          nc.vector.tensor_copy(out=hT[:], in_=hT_ps[:])
                          nc.tensor.matmul(h2_ps[:], lhsT=hT[:],
                                           rhs=w2_sb[:, k, :],
                                           start=(k == 0), stop=(k == 1))
                      t2t = sp2.tile([P, ROW2E], F16)
                      tmp = sp2.tile([P, HID], F32)
                      red = sp2.tile([P, 1], F32)
                      nc.vector.tensor_tensor(out=tmp[:], in0=h2_ps[:],
                                              in1=as2_sb[:], op=A.mult)
                      nc.vector.tensor_reduce(out=red[:], in_=tmp[:],
                                              axis=X, op=A.add)
                      nc.vector.tensor_copy(out=t2t[:, OFF2:OFF2 + 1],
                                            in_=red[:])
                      nc.vector.tensor_tensor(out=tmp[:], in0=h2_ps[:],
                                              in1=ad2_sb[:], op=A.mult)
                      nc.vector.tensor_reduce(out=red[:], in_=tmp[:],
                                              axis=X, op=A.add)
                      nc.vector.tensor_copy(out=adst2_sb[:, t, :], in_=red[:])
                      nc.vector.tensor_copy(out=t2t[:, 0:HID], in_=h2_ps[:])
                      nc.vector.memset(t2t[:, OFF2 + 1:ROW2E], 0.0)
                      nc.sync.dma_start(t2loc[t * P:(t + 1) * P, :], t2t[:])
                      if dbg:
                          nc.sync.dma_start(t2dbg_d[t * P:(t + 1) * P, :],
                                            t2t[:])
                  nc.gpsimd.collective_compute(
                      "AllGather", A.bypass, replica_groups=rg,
                      ins=[t2loc[:]], outs=[t2full[:]])

            # ---------------- Phase 3: layer-2 edge phase + pooling ---------
            with tc.tile_pool(name="p3g", bufs=2) as pg, \
                 tc.tile_pool(name="p3r", bufs=2) as pr, \
                 tc.tile_pool(name="p3w", bufs=2) as pw, \
                 tc.tile_pool(name="p3oh", bufs=2) as poh, \
                 tc.tile_pool(name="p3c", bufs=1) as pc, \
                 tc.tile_pool(name="p3ps", bufs=2, space="PSUM") as pps, \
                 tc.tile_pool(name="p3ps2", bufs=2, space="PSUM") as pps2, \
                 tc.tile_pool(name="p3pl", bufs=1, space="PSUM") as ppl:
                b2_sb = pc.tile([P, HID], F32)
                nc.sync.dma_start(b2_sb[:], b2_d[:])
                pool_ps = ppl.tile([GPC, HID + 1], F32, space="PSUM")
                for t in range(NT):
                    acc = pps.tile([P, HID + 1], F32, space="PSUM")
                    gbase = t * NIc
                    cbase = t * CPT
                    G = pg.tile([P, CPT, ROW2E], F16)
                    gather(G, t2full[:], gbase, NI, ROW2E,
                           first=(t == 0))
                    dr = pr.tile([P, CPT, P], F16)
                    nc.sync.dma_start(
                        dr[:],
                        dstlr_d[:, cbase * P:(cbase + CPT) * P]
                        .rearrange("p (c e) -> p c e", c=CPT))
                    ohT = poh.tile([P, CPT, P], F16)
                    nc.vector.tensor_scalar(
                        out=ohT[:], in0=dr[:],
                        scalar1=pidx_sb[:, 0:1], scalar2=None,
                        op0=A.is_equal)
                    oh = poh.tile([P, CPT, P], F16)
                    nc.vector.tensor_tensor(
                        out=oh[:],
                        in0=iota16_sb[:].unsqueeze(1)
                            .to_broadcast([P, CPT, P]),
                        in1=dstl_sb[:, cbase:cbase + CPT]
                            .unsqueeze(2).to_broadcast([P, CPT, P]),
                        op=A.is_equal)
                    adst_ps = pps2.tile([P, CPT, 1], F32, space="PSUM")
                    for c in range(CPT):
                        nc.tensor.matmul(
                            adst_ps[:, c, :], lhsT=ohT[:, c, :],
                            rhs=adst2_sb[:, t, :],
                            start=True, stop=True,
                            skip_group_check=True)
                    asr = G[:, :, OFF2:OFF2 + 1]
                    adst16 = pw.tile([P, CPT, 1], F16)
                    nc.vector.tensor_copy(out=adst16[:], in_=adst_ps[:])
                    ae = pw.tile([P, CPT], F16)
                    nc.vector.tensor_scalar(
                        out=ae[:], in0=wv_sb[:, cbase:cbase + CPT],
                        scalar1=q2_sb[:, 0:1], scalar2=None, op0=A.mult)
                    nc.vector.tensor_tensor(out=ae[:], in0=ae[:],
                                            in1=adst16[:, :, 0],
                                            op=A.add)
                    nc.vector.tensor_tensor(out=asr, in0=asr,
                                            in1=ae[:].unsqueeze(2),
                                            op=A.add)
                    e2 = pw.tile([P, CPT, 1], F16)
                    nc.scalar.activation(out=e2[:], in_=asr,
                                         func=ACT.Exp, scale=0.2)
                    nc.scalar.activation(out=asr, in_=asr, func=ACT.Exp)
                    nc.vector.tensor_tensor(out=asr, in0=asr, in1=e2[:],
                                            op=A.max)
                    gm = G[:, :, 0:HID]
                    nc.vector.tensor_tensor(
                        out=gm, in0=gm,
                        in1=asr.to_broadcast([P, CPT, HID]), op=A.mult)
                    for c in range(CPT):
                        nc.tensor.matmul(
                            acc[:], lhsT=oh[:, c, :],
                            rhs=G[:, c, 0:HID + 1],
                            start=(c == 0),
                            stop=(c == CPT - 1))
                    # epilogue: o2 = [relu(acc/denom + b2) | 1]
                    dn = pw.tile([P, 1], F32)
                    nc.vector.tensor_scalar(out=dn[:], in0=acc[:, HID:HID + 1],
                                            scalar1=1e-16, scalar2=None,
                                            op0=A.add)
                    rc = pw.tile([P, 1], F32)
                    nc.vector.reciprocal(rc[:], dn[:])
                    o2 = pw.tile([P, HID + 1], F32)
                    nc.vector.tensor_scalar(out=o2[:, 0:HID],
                                            in0=acc[:, 0:HID],
                                            scalar1=rc[:, 0:1], scalar2=None,
                                            op0=A.mult)
                    nc.vector.tensor_tensor(out=o2[:, 0:HID],
                                            in0=o2[:, 0:HID],
                                            in1=b2_sb[:], op=A.add)
                    nc.vector.tensor_scalar(out=o2[:, 0:HID],
                                            in0=o2[:, 0:HID],
                                            scalar1=0.0, scalar2=None,
                                            op0=A.max)
                    nc.vector.memset(o2[:, HID:HID + 1], 1.0)
                    ohg = poh.tile([P, GPC], F32)
                    nc.vector.tensor_scalar(
                        out=ohg[:], in0=iota_sb[:, 0:GPC],
                        scalar1=gl_sb[:, t:t + 1], scalar2=None,
                        op0=A.is_equal)
                    nc.tensor.matmul(pool_ps[:], lhsT=ohg[:], rhs=o2[:],
                                     start=(t == 0), stop=(t == NT - 1),
                                     skip_group_check=True)

                # ------------- Phase 4: pooled mean + FC --------------------
                fcw_sb = pc.tile([HID, OUT], F32)
                nc.sync.dma_start(fcw_sb[:], fcw_d[:])
                fcb_sb = pc.tile([P, OUT], F32)
                nc.sync.dma_start(fcb_sb[:], fcb_d[:])
                if dbg:
                    plt_sb = pc.tile([GPC, HID + 1], F32)
                    nc.vector.tensor_copy(out=plt_sb[:], in_=pool_ps[:])
                    nc.sync.dma_start(pldbg_d[:], plt_sb[:])
                cnt = pc.tile([GPC, 1], F32)
                nc.vector.tensor_scalar(out=cnt[:],
                                        in0=pool_ps[:, HID:HID + 1],
                                        scalar1=1.0, scalar2=None, op0=A.max)
                rcc = pc.tile([GPC, 1], F32)
                nc.vector.reciprocal(rcc[:], cnt[:])
                pooled = pc.tile([GPC, HID], F32)
                nc.vector.tensor_scalar(out=pooled[:], in0=pool_ps[:, 0:HID],
                                        scalar1=rcc[:, 0:1], scalar2=None,
                                        op0=A.mult)
                pT_ps = ppl.tile([HID, GPC], F32, space="PSUM")
                nc.tensor.transpose(pT_ps[:], pooled[:], ident32[:GPC, :GPC])
                pT = pc.tile([HID, GPC], F32)
                nc.vector.tensor_copy(out=pT[:], in_=pT_ps[:])
                fc_ps = ppl.tile([GPC, OUT], F32, space="PSUM")
                nc.tensor.matmul(fc_ps[:], lhsT=pT[:], rhs=fcw_sb[:],
                                 start=True, stop=True)
                res = pc.tile([GPC, OUT], F32)
                nc.vector.tensor_tensor(out=res[:], in0=fc_ps[:],
                                        in1=fcb_sb[:GPC, :], op=A.add)
                nc.sync.dma_start(out_d[:], res[:])

    nc.compile()
    return nc


# ---------------------------------------------------------------------------
# Entry point.
# ---------------------------------------------------------------------------
def run(inputs, cfg, **run_kwargs):
    in_maps, meta = prepare(inputs, cfg)
    nc = build(meta)
    res = run_bass_kernel_spmd(nc, in_maps, core_ids=list(range(NCORES)),
                               **run_kwargs)
    out = np.concatenate([res.results[c]["out"] for c in range(NCORES)],
                         axis=0)
    return np.asarray(out, np.float32), res


def kernel(**inputs) -> np.ndarray:
    out, _ = run(inputs, FULL_CFG)
    return out


# revision 19
# speedup vs baseline: 1.1360x; 1.1360x over previous
"""Trainium2 Bass kernel for a 2-layer edge-featured GAT + mean-pool + FC.

Sharding: 256 graphs are split 32-per-core across 8 cores. Because `batch` is
sorted, each core owns a contiguous node range (graph-aligned), so both the
per-destination softmax segments and the mean-pool segments are core-local.
Edges are assigned to the core that owns their destination node. Between
layers, per-core node tables (features + src-attention logits, fp16) are
AllGathered so every core can gather arbitrary source rows.

Edge phase (per layer): edges sorted by dst are packed into 128-edge chunks
grouped by destination node tile (128 nodes). Per half-tile group of CH
chunks, a single dma_gather (prepare_only + trigger_dma on a rotating SWDGE
queue, so descriptor generation pipelines with DMA drains) pulls fp16
source-node rows [h | a_src | pad]. The dst-side a_dst term needs no gather:
it is taken from the core-local a_dst table via a transposed one-hot matmul
(ohT[d,e] @ a_dst_tile[d,:]). Attention weights are
p = exp(leaky_relu(a_src+a_dst+w*q)) computed as max(exp(x), exp(0.2x)),
messages are h*p, and the segment-sum over destinations is a one-hot matmul
accumulated in PSUM — with p itself carried as extra columns to produce the
softmax denominators. Softmax normalization happens once per node after
aggregation: out = (sum p*h) / (sum p + 1e-16), exactly equivalent to the
reference's per-edge normalization (the max-subtraction cancels in the
ratio). Pad edge slots gather row 0 (finite data) and carry dst_local = -1
so their one-hot columns are all zero and they contribute nothing.
"""

import sys

sys.path.insert(0, "/opt/trn_rl_repo")

import math
from contextlib import ExitStack

import numpy as np

import concourse.bacc as bacc
import concourse.bass as bass
import concourse.mybir as mybir
import concourse.tile as tile
from concourse.bass_utils import run_bass_kernel_spmd
from concourse.masks import make_identity

P = 128
NCORES = 8
NQ = 4  # SWDGE queues
SP = False  # dma_gather single_packet (True crashes NRT)
PREP = False  # prepare_only path broken on HW

FULL_CFG = dict(N=20000, E=640000, FIN=128, HID=64, HEADS=4, NG=256, OUT=32)

F32 = mybir.dt.float32
F16 = mybir.dt.float16
I16 = mybir.dt.int16

ROW1E = 384  # fp16 elems per layer-1 table row: h(256) asrc(4) pad(124)
ROW2E = 128  # fp16 elems per layer-2 table row: h2(64) asrc2(1) pad(63)


# ---------------------------------------------------------------------------
# Host-side preparation: integer index manipulation + array reordering only.
# ---------------------------------------------------------------------------
def prepare(inputs, cfg):
    N, E, FIN, HID, HEADS, NG, OUT = (
        cfg["N"], cfg["E"], cfg["FIN"], cfg["HID"], cfg["HEADS"], cfg["NG"],
        cfg["OUT"],
    )
    GPC = NG // NCORES  # graphs per core

    x = np.asarray(inputs["x"], np.float32)
    ei = np.asarray(inputs["edge_index"], np.int64)
    ea = np.asarray(inputs["edge_attr"], np.float32)
    batch = np.asarray(inputs["batch"], np.int64)
    src, dst = ei[0], ei[1]

    # node ranges per core (graph-aligned; batch is sorted)
    bounds = np.searchsorted(batch, np.arange(NCORES + 1) * GPC)
    node_cnt = np.diff(bounds)
    NT = max(1, math.ceil(node_cnt.max() / P))
    NSLICE = NT * P
    NROWS = NCORES * NSLICE
    assert NROWS < 32768, f"int16 gather index overflow: {NROWS}"

    core_of_node = np.minimum(batch // GPC, NCORES - 1).astype(np.int64)
    rowid = np.empty(N, np.int64)
    for c in range(NCORES):
        ns, ne = bounds[c], bounds[c + 1]
        rowid[ns:ne] = c * NSLICE + np.arange(ne - ns)

    # edges sorted by dst; since batch is sorted, core blocks are contiguous
    order = np.argsort(dst, kind="stable")
    dsts = dst[order]
    srcs = src[order]
    ws = ea[order, 0]
    ecore = core_of_node[dsts]
    ebounds = np.searchsorted(ecore, np.arange(NCORES + 1))

    # chunks-per-tile: max over all (core, tile), rounded up to even
    cpt_max = 1
    tile_edge_counts = []
    for c in range(NCORES):
        es, ee = ebounds[c], ebounds[c + 1]
        dln = dsts[es:ee] - bounds[c]
        tid = dln // P
        cnts = np.bincount(tid, minlength=NT)
        tile_edge_counts.append(cnts)
        if len(cnts):
            cpt_max = max(cpt_max, math.ceil(cnts.max() / P))
    CPT = cpt_max + (cpt_max % 2)  # even
    CPT = max(CPT, 2)
    CH = CPT // 2
    NCHUNK = NT * CPT

    per_core = []
    for c in range(NCORES):
        ns, ne = bounds[c], bounds[c + 1]
        es, ee = ebounds[c], ebounds[c + 1]
        nloc = ne - ns

        xs = np.zeros((NSLICE, FIN), np.float16)
        xs[:nloc] = x[ns:ne].astype(np.float16)

        gl = np.full((NT * P,), -1.0, np.float32)
        gl[:nloc] = (batch[ns:ne] - c * GPC).astype(np.float32)
        gl_dev = gl.reshape(NT, P).T.copy()  # [128, NT]

        srcrow = np.zeros((NT, CPT * P), np.int64)
        dstl = np.full((NT, CPT * P), -1.0, np.float32)
        wv = np.zeros((NT, CPT * P), np.float32)

        dln = dsts[es:ee] - ns
        tid = dln // P
        cnts = tile_edge_counts[c]
        off = np.zeros(NT + 1, np.int64)
        off[1:NT + 1] = np.cumsum(cnts[:NT])
        for t in range(NT):
            k = int(cnts[t]) if t < len(cnts) else 0
            if k == 0:
                continue
            sel = slice(es + int(off[t]), es + int(off[t]) + k)
            srcrow[t, :k] = rowid[srcs[sel]]
            dstl[t, :k] = (dln[int(off[t]):int(off[t]) + k] % P).astype(
                np.float32)
            wv[t, :k] = ws[sel]

        # device layouts
        dstl_dev = dstl.reshape(NCHUNK, P).T.astype(np.float16).copy()
        wv_dev = wv.reshape(NCHUNK, P).T.astype(np.float16).copy()
        # dstl replicated across partitions: [128, NCHUNK*P], row-major per
        # chunk, every partition identical (for the transposed one-hot build)
        dstl_rep = np.tile(
            dstl.reshape(1, NCHUNK * P), (P, 1)).astype(np.float16)

        def wrap_idx(arr):  # [NT, CPT*P] -> [128, NT*CPT*8] int16
            blocks = []
            for t in range(NT):
                for h in range(2):
                    ids = arr[t, h * CH * P:(h + 1) * CH * P]
                    a = ids.reshape(CH * 8, 16).T  # [16, CH*8]
                    blocks.append(np.tile(a, (8, 1)))
            return np.ascontiguousarray(
                np.concatenate(blocks, axis=1)).astype(np.int16)

        per_core.append(dict(
            xs=xs, gl=gl_dev, dstl=dstl_dev, dstlr=dstl_rep, wv=wv_dev,
            idxs=wrap_idx(srcrow),
        ))

    # weight-side constants (tiny, host-replicated)
    W1 = np.asarray(inputs["W1"], np.float32).astype(np.float16)  # [FIN,H*HID]
    W2 = np.asarray(inputs["W2"], np.float32).astype(np.float16)  # [H*HID,HID]
    as1 = np.asarray(inputs["att_src1"], np.float32).reshape(-1)
    ad1 = np.asarray(inputs["att_dst1"], np.float32).reshape(-1)
    as2 = np.asarray(inputs["att_src2"], np.float32).reshape(-1)
    ad2 = np.asarray(inputs["att_dst2"], np.float32).reshape(-1)
    q1 = (np.asarray(inputs["We1"], np.float32).reshape(HEADS, HID)
          * np.asarray(inputs["att_edge1"], np.float32)).sum(axis=1)  # [H]
    q2 = float((np.asarray(inputs["We2"], np.float32).reshape(-1)
                * np.asarray(inputs["att_edge2"], np.float32).reshape(-1))
               .sum())
    b1 = np.asarray(inputs["b1"], np.float32)
    b2 = np.asarray(inputs["b2"], np.float32)
    fcW = np.asarray(inputs["fcW"], np.float32)
    fcb = np.asarray(inputs["fcb"], np.float32)

    rep = lambda vv: np.tile(vv[None, :].astype(np.float32), (P, 1)).copy()
    rep16 = lambda vv: np.tile(vv[None, :].astype(np.float16), (P, 1)).copy()
    consts = dict(
        W1=W1, W2=W2,
        as1b=rep(as1), ad1b=rep(ad1), b1b=rep16(b1),
        as2b=rep(as2), ad2b=rep(ad2), b2b=rep(b2),
        q1b=rep16(q1), q2b=np.full((P, 1), q2, np.float32),
        fcw=fcW, fcbb=rep(fcb),
        iota=np.tile(np.arange(P, dtype=np.float32)[None, :], (P, 1)).copy(),
        iota16=np.tile(np.arange(P, dtype=np.float16)[None, :], (P, 1)).copy(),
        pidx32=np.arange(P, dtype=np.float32)[:, None].copy(),
    )

    in_maps = []
    for c in range(NCORES):
        m = dict(per_core[c])
        m.update(consts)
        in_maps.append(m)

    meta = dict(NT=NT, CPT=CPT, CH=CH, NSLICE=NSLICE, NROWS=NROWS,
                GPC=GPC, **cfg)
    return in_maps, meta


# ---------------------------------------------------------------------------
# Device program.
# ---------------------------------------------------------------------------
def build(meta, reps=1, num_devices=NCORES):
    NT, CPT, CH = meta["NT"], meta["CPT"], meta["CH"]
    NSLICE, NROWS, GPC = meta["NSLICE"], meta["NROWS"], meta["GPC"]
    FIN, HID, HEADS, OUT = meta["FIN"], meta["HID"], meta["HEADS"], meta["OUT"]
    D1 = HEADS * HID          # 256
    OFF1 = D1                 # asrc offset in row1
    OFF2 = HID                # asrc offset in row2
    NI = CH * P               # idxs per gather group
    NIc = NI // 16            # idx columns per group
    NCHUNK = NT * CPT
    A = mybir.AluOpType
    ACT = mybir.ActivationFunctionType
    X = mybir.AxisListType.X
    rg = [list(range(NCORES))]

    nc = bacc.Bacc("TRN2", target_bir_lowering=False, debug=False,
                   num_devices=num_devices,
                   dynamic_dma_scratch_size=65536,
                   num_swdge_queues=NQ)

    dma_sems = [nc.alloc_semaphore(f"gsem{q}") for q in range(NQ)]

    def din(name, shape, dtype=F32):
        return nc.dram_tensor(name, list(shape), dtype,
                              kind="ExternalInput").ap()

    xs = din("xs", (NSLICE, FIN), F16)
    idxs_d = din("idxs", (P, NT * CPT * 8), I16)
    dstl_d = din("dstl", (P, NCHUNK), F16)
    dstlr_d = din("dstlr", (P, NCHUNK * P), F16)
    wv_d = din("wv", (P, NCHUNK), F16)
    gl_d = din("gl", (P, NT))
    W1_d = din("W1", (FIN, D1), F16)
    W2_d = din("W2", (D1, HID), F16)
    as1_d = din("as1b", (P, D1))
    ad1_d = din("ad1b", (P, D1))
    b1_d = din("b1b", (P, D1), F16)
    as2_d = din("as2b", (P, HID))
    ad2_d = din("ad2b", (P, HID))
    b2_d = din("b2b", (P, HID))
    q1_d = din("q1b", (P, HEADS), F16)
    q2_d = din("q2b", (P, 1))
    fcw_d = din("fcw", (HID, OUT))
    fcb_d = din("fcbb", (P, OUT))
    iota_d = din("iota", (P, P))
    iota16_d = din("iota16", (P, P), F16)
    pidx32_d = din("pidx32", (P, 1), F32)

    out_d = nc.dram_tensor("out", [GPC, OUT], F32, kind="ExternalOutput").ap()
    dbg = meta.get("debug")
    if dbg:
        t1dbg_d = nc.dram_tensor("t1dbg", [NSLICE, ROW1E], F16,
                                 kind="ExternalOutput").ap()
        o1dbg_d = nc.dram_tensor("o1dbg", [P, NT * D1], F16,
                                 kind="ExternalOutput").ap()
        ad1dbg_d = nc.dram_tensor("ad1dbg", [P, NT * HEADS], F16,
                                  kind="ExternalOutput").ap()
        t2dbg_d = nc.dram_tensor("t2dbg", [NSLICE, ROW2E], F16,
                                 kind="ExternalOutput").ap()
        pldbg_d = nc.dram_tensor("pldbg", [GPC, HID + 1], F32,
                                 kind="ExternalOutput").ap()

    with tile.TileContext(nc) as tc, ExitStack() as st:
        constp = st.enter_context(tc.tile_pool(name="constp", bufs=1))
        drp = st.enter_context(tc.tile_pool(name="drp", bufs=1, space="DRAM"))

        # whole-kernel constants
        iota_sb = constp.tile([P, P], F32)
        nc.sync.dma_start(iota_sb[:], iota_d[:])
        iota16_sb = constp.tile([P, P], F16)
        nc.sync.dma_start(iota16_sb[:], iota16_d[:])
        pidx_sb = constp.tile([P, 1], F32)
        nc.sync.dma_start(pidx_sb[:], pidx32_d[:])
        ident16 = constp.tile([P, P], F16)
        nc.vector.tensor_scalar(out=ident16[:], in0=iota16_sb[:],
                                scalar1=pidx_sb[:, 0:1], scalar2=None,
                                op0=A.is_equal)
        ident32 = constp.tile([P, P], F32)
        make_identity(nc, ident32[:])
        dstl_sb = constp.tile([P, NCHUNK], F16)
        nc.sync.dma_start(dstl_sb[:], dstl_d[:])
        wv_sb = constp.tile([P, NCHUNK], F16)
        nc.sync.dma_start(wv_sb[:], wv_d[:])
        gl_sb = constp.tile([P, NT], F32)
        nc.sync.dma_start(gl_sb[:], gl_d[:])
        q1_sb = constp.tile([P, HEADS], F16)
        nc.sync.dma_start(q1_sb[:], q1_d[:])
        q2_sb = constp.tile([P, 1], F32)
        nc.sync.dma_start(q2_sb[:], q2_d[:])
        ixs_all = constp.tile([P, NT * CPT * 8], I16)
        nc.sync.dma_start(ixs_all[:], idxs_d[:])
        adst1_sb = constp.tile([P, NT, HEADS], F16)
        adst2_sb = constp.tile([P, NT, 1], F16)

        qctr = [0]

        def gather(G, table, gbase, ni, elem, first=False):
            q = qctr[0] % NQ
            qctr[0] += 1
            if PREP and not first:
                nc.gpsimd.dma_gather(
                    G[:], table, ixs_all[:, gbase:gbase + NIc],
                    ni, ni, elem, single_packet=SP,
                    prepare_only=True, sem=dma_sems[q], queue_num=q)
                nc.gpsimd.trigger_dma(count=None, queue_num=q)
            else:
                nc.gpsimd.dma_gather(
                    G[:], table, ixs_all[:, gbase:gbase + NIc],
                    ni, ni, elem, single_packet=SP, queue_num=q)

        for _rep in range(reps):
            t1loc = drp.tile([NSLICE, ROW1E], F16, name=f"t1loc{_rep}")
            t1full = drp.tile([NROWS, ROW1E], F16, addr_space="Shared",
                              name=f"t1full{_rep}")
            t2loc = drp.tile([NSLICE, ROW2E], F16, name=f"t2loc{_rep}")
            t2full = drp.tile([NROWS, ROW2E], F16, addr_space="Shared",
                              name=f"t2full{_rep}")

            # ---------------- Phase 0: h1 = x @ W1, a_src/a_dst, table1 -----
            with tc.tile_pool(name="ph0", bufs=1) as sp, \
                 tc.tile_pool(name="ph0b", bufs=2) as sp2, \
                 tc.tile_pool(name="ph0p", bufs=2, space="PSUM") as pp:
                w1_sb = sp.tile([P, D1], F16)
                nc.sync.dma_start(w1_sb[:], W1_d[:])
                as1_sb = sp.tile([P, D1], F32)
                nc.sync.dma_start(as1_sb[:], as1_d[:])
                ad1_sb = sp.tile([P, D1], F32)
                nc.sync.dma_start(ad1_sb[:], ad1_d[:])
                xall = sp.tile([P, NT, FIN], F16)
                nc.sync.dma_start(xall[:],
                                  xs[:].rearrange("(t p) f -> p t f", p=P))
                for t in range(NT):
                    xT_ps = pp.tile([P, P], F16, space="PSUM")
                    nc.tensor.transpose(xT_ps[:], xall[:, t, :], ident16[:])
                    xT = sp2.tile([P, P], F16)
                    nc.vector.tensor_copy(out=xT[:], in_=xT_ps[:])
                    h_ps = pp.tile([P, D1], F32, space="PSUM")
                    nc.tensor.matmul(h_ps[:], lhsT=xT[:], rhs=w1_sb[:],
                                     start=True, stop=True)
                    t1t = sp2.tile([P, ROW1E], F16)
                    tmp = sp2.tile([P, D1], F32)
                    red = sp2.tile([P, HEADS], F32)
                    nc.vector.tensor_tensor(out=tmp[:], in0=h_ps[:],
                                            in1=as1_sb[:], op=A.mult)
                    nc.vector.tensor_reduce(
                        out=red[:],
                        in_=tmp[:].rearrange("p (h f) -> p h f", h=HEADS),
                        axis=X, op=A.add)
                    nc.vector.tensor_copy(out=t1t[:, OFF1:OFF1 + HEADS],
                                          in_=red[:])
                    nc.vector.tensor_tensor(out=tmp[:], in0=h_ps[:],
                                            in1=ad1_sb[:], op=A.mult)
                    nc.vector.tensor_reduce(
                        out=red[:],
                        in_=tmp[:].rearrange("p (h f) -> p h f", h=HEADS),
                        axis=X, op=A.add)
                    nc.vector.tensor_copy(out=adst1_sb[:, t, :], in_=red[:])
                    nc.vector.tensor_copy(out=t1t[:, 0:D1], in_=h_ps[:])
                    nc.vector.memset(t1t[:, OFF1 + HEADS:ROW1E], 0.0)
                    nc.sync.dma_start(t1loc[t * P:(t + 1) * P, :], t1t[:])
                    if dbg:
                        nc.sync.dma_start(t1dbg_d[t * P:(t + 1) * P, :],
                                          t1t[:])
                nc.gpsimd.collective_compute(
                    "AllGather", A.bypass, replica_groups=rg,
                    ins=[t1loc[:]], outs=[t1full[:]])

            # ---------------- Phase 1: layer-1 edge phase -------------------
            with tc.tile_pool(name=f"outp{_rep}", bufs=1) as outp:
              out1 = outp.tile([P, NT * D1], F16, name=f"out1_{_rep}")
              b1_sb = outp.tile([P, D1], F16)
              nc.sync.dma_start(b1_sb[:], b1_d[:])
              w2_sb = outp.tile([P, 2, HID], F16)
              nc.sync.dma_start(w2_sb[:],
                                W2_d[:].rearrange("(k p) n -> p k n", p=P))
              as2_sb = outp.tile([P, HID], F32)
              nc.sync.dma_start(as2_sb[:], as2_d[:])
              ad2_sb = outp.tile([P, HID], F32)
              nc.sync.dma_start(ad2_sb[:], ad2_d[:])
              with tc.tile_pool(name="p1g", bufs=3) as pg, \
                   tc.tile_pool(name="p1r", bufs=3) as pr, \
                   tc.tile_pool(name="p1w", bufs=2) as pw, \
                   tc.tile_pool(name="p1oh", bufs=3) as poh, \
                   tc.tile_pool(name="p1ps", bufs=2, space="PSUM") as pps, \
                   tc.tile_pool(name="p1ps2", bufs=2, space="PSUM") as pps2:
                  for t in range(NT):
                      acc = pps.tile([P, D1 + HEADS], F32, space="PSUM")
                      for hh in range(2):
                          gbase = (t * 2 + hh) * NIc
                          cbase = (t * 2 + hh) * CH
                          G = pg.tile([P, CH, ROW1E], F16)
                          gather(G, t1full[:], gbase, NI, ROW1E,
                                 first=(t == 0 and hh == 0))
                          dr = pr.tile([P, CH, P], F16)
                          nc.sync.dma_start(
                              dr[:],
                              dstlr_d[:, cbase * P:(cbase + CH) * P]
                              .rearrange("p (c e) -> p c e", c=CH))
                          ohT = poh.tile([P, CH, P], F16)
                          nc.vector.tensor_scalar(
                              out=ohT[:], in0=dr[:],
                              scalar1=pidx_sb[:, 0:1], scalar2=None,
                              op0=A.is_equal)
                          oh = poh.tile([P, CH, P], F16)
                          nc.vector.tensor_tensor(
                              out=oh[:],
                              in0=iota16_sb[:].unsqueeze(1)
                                  .to_broadcast([P, CH, P]),
                              in1=dstl_sb[:, cbase:cbase + CH]
                                  .unsqueeze(2).to_broadcast([P, CH, P]),
                              op=A.is_equal)
                          adst_ps = pps2.tile([P, CH, HEADS], F32,
                                              space="PSUM")
                          for c in range(CH):
                              nc.tensor.matmul(
                                  adst_ps[:, c, :], lhsT=ohT[:, c, :],
                                  rhs=adst1_sb[:, t, :],
                                  start=True, stop=True,
                                  skip_group_check=True)
                          asr = G[:, :, OFF1:OFF1 + HEADS]
                          adst16 = pw.tile([P, CH, HEADS], F16)
                          nc.vector.tensor_copy(out=adst16[:], in_=adst_ps[:])
                          ae = pw.tile([P, CH, HEADS], F16)
                          nc.vector.tensor_tensor(
                              out=ae[:],
                              in0=wv_sb[:, cbase:cbase + CH].unsqueeze(2)
                                  .to_broadcast([P, CH, HEADS]),
                              in1=q1_sb[:].unsqueeze(1)
                                  .to_broadcast([P, CH, HEADS]),
                              op=A.mult)
                          nc.vector.tensor_tensor(out=ae[:], in0=ae[:],
                                                  in1=adst16[:], op=A.add)
                          nc.vector.tensor_tensor(out=asr, in0=asr,
                                                  in1=ae[:], op=A.add)
                          e2 = pw.tile([P, CH, HEADS], F16)
                          nc.scalar.activation(out=e2[:], in_=asr,
                                               func=ACT.Exp, scale=0.2)
                          nc.scalar.activation(out=asr, in_=asr, func=ACT.Exp)
                          nc.vector.tensor_tensor(out=asr, in0=asr, in1=e2[:],
                                                  op=A.max)
                          gm = G[:, :, 0:D1].rearrange(
                              "p c (h f) -> p c h f", h=HEADS)
                          nc.vector.tensor_tensor(
                              out=gm, in0=gm,
                              in1=asr.unsqueeze(3)
                                  .to_broadcast([P, CH, HEADS, HID]),
                              op=A.mult)
                          for c in range(CH):
                              nc.tensor.matmul(
                                  acc[:], lhsT=oh[:, c, :],
                                  rhs=G[:, c, 0:D1 + HEADS],
                                  start=(hh == 0 and c == 0),
                                  stop=(hh == 1 and c == CH - 1))
                      # epilogue: out1 = relu(acc/denom + b1)
                      dn = pw.tile([P, HEADS], F32)
                      nc.vector.tensor_scalar(out=dn[:],
                                              in0=acc[:, D1:D1 + HEADS],
                                              scalar1=1e-16, scalar2=None,
                                              op0=A.add)
                      rc = pw.tile([P, HEADS], F32)
                      nc.vector.reciprocal(rc[:], dn[:])
                      ob = out1[:, t * D1:(t + 1) * D1]
                      nc.vector.tensor_tensor(
                          out=ob.rearrange("p (h f) -> p h f", h=HEADS),
                          in0=acc[:, 0:D1].rearrange("p (h f) -> p h f",
                                                     h=HEADS),
                          in1=rc[:].unsqueeze(2)
                              .to_broadcast([P, HEADS, HID]),
                          op=A.mult)
                      nc.vector.tensor_tensor(out=ob, in0=ob, in1=b1_sb[:],
                                              op=A.add)
                      nc.vector.tensor_scalar(out=ob, in0=ob, scalar1=0.0,
                                              scalar2=None, op0=A.max)
                      # phase-2 for this tile: h2 = relu(out1_t) @ W2
                      h2_ps = pps2.tile([P, HID], F32, space="PSUM")
                      for k in range(2):
                          hT_ps = pps2.tile([P, P], F16, space="PSUM")
                          nc.tensor.transpose(
                              hT_ps[:],
                              out1[:, t * D1 + k * P:t * D1 + (k + 1) * P],
                              ident16[:])
                          hT = pw.tile([P, P], F16)
                          nc.vector.tensor_copy(out=hT[:], in_=hT_ps[:])
                          nc.tensor.matmul(h2_ps[:], lhsT=hT[:],
                                           rhs=w2_sb[:, k, :],
                                           start=(k == 0), stop=(k == 1))
                      t2t = pw.tile([P, ROW2E], F16)
                      tmp2 = pw.tile([P, HID], F32)
                      red2 = pw.tile([P, 1], F32)
                      nc.vector.tensor_tensor(out=tmp2[:], in0=h2_ps[:],
                                              in1=as2_sb[:], op=A.mult)
                      nc.vector.tensor_reduce(out=red2[:], in_=tmp2[:],
                                              axis=X, op=A.add)
                      nc.vector.tensor_copy(out=t2t[:, OFF2:OFF2 + 1],
                                            in_=red2[:])
                      nc.vector.tensor_tensor(out=tmp2[:], in0=h2_ps[:],
                                              in1=ad2_sb[:], op=A.mult)
                      nc.vector.tensor_reduce(out=red2[:], in_=tmp2[:],
                                              axis=X, op=A.add)
                      nc.vector.tensor_copy(out=adst2_sb[:, t, :],
                                            in_=red2[:])
                      nc.vector.tensor_copy(out=t2t[:, 0:HID], in_=h2_ps[:])
                      nc.vector.memset(t2t[:, OFF2 + 1:ROW2E], 0.0)
                      nc.sync.dma_start(t2loc[t * P:(t + 1) * P, :], t2t[:])

              if dbg:
                  nc.sync.dma_start(o1dbg_d[:], out1[:])
                  nc.sync.dma_start(
                      ad1dbg_d[:],
                      adst1_sb[:].rearrange("p t h -> p (t h)"))

              # ---------------- Phase 2 (fused above): AllGather table2 -----
              nc.gpsimd.collective_compute(
                  "AllGather", A.bypass, replica_groups=rg,
                  ins=[t2loc[:]], outs=[t2full[:]])

            # ---------------- Phase 3: layer-2 edge phase + pooling ---------
            with tc.tile_pool(name="p3g", bufs=3) as pg, \
                 tc.tile_pool(name="p3r", bufs=3) as pr, \
                 tc.tile_pool(name="p3w", bufs=2) as pw, \
                 tc.tile_pool(name="p3oh", bufs=3) as poh, \
                 tc.tile_pool(name="p3c", bufs=1) as pc, \
                 tc.tile_pool(name="p3ps", bufs=2, space="PSUM") as pps, \
                 tc.tile_pool(name="p3ps2", bufs=2, space="PSUM") as pps2, \
                 tc.tile_pool(name="p3pl", bufs=1, space="PSUM") as ppl:
                b2_sb = pc.tile([P, HID], F32)
                nc.sync.dma_start(b2_sb[:], b2_d[:])
                pool_ps = ppl.tile([GPC, HID + 1], F32, space="PSUM")
                for t in range(NT):
                    acc = pps.tile([P, HID + 1], F32, space="PSUM")
                    for hh in range(2):
                        gbase = (t * 2 + hh) * NIc
                        cbase = (t * 2 + hh) * CH
                        G = pg.tile([P, CH, ROW2E], F16)
                        gather(G, t2full[:], gbase, NI, ROW2E,
                               first=(t == 0 and hh == 0))
                        dr = pr.tile([P, CH, P], F16)
                        nc.sync.dma_start(
                            dr[:],
                            dstlr_d[:, cbase * P:(cbase + CH) * P]
                            .rearrange("p (c e) -> p c e", c=CH))
                        ohT = poh.tile([P, CH, P], F16)
                        nc.vector.tensor_scalar(
                            out=ohT[:], in0=dr[:],
                            scalar1=pidx_sb[:, 0:1], scalar2=None,
                            op0=A.is_equal)
                        oh = poh.tile([P, CH, P], F16)
                        nc.vector.tensor_tensor(
                            out=oh[:],
                            in0=iota16_sb[:].unsqueeze(1)
                                .to_broadcast([P, CH, P]),
                            in1=dstl_sb[:, cbase:cbase + CH]
                                .unsqueeze(2).to_broadcast([P, CH, P]),
                            op=A.is_equal)
                        adst_ps = pps2.tile([P, CH, 1], F32, space="PSUM")
                        for c in range(CH):
                            nc.tensor.matmul(
                                adst_ps[:, c, :], lhsT=ohT[:, c, :],
                                rhs=adst2_sb[:, t, :],
                                start=True, stop=True,
                                skip_group_check=True)
                        asr = G[:, :, OFF2:OFF2 + 1]
                        adst16 = pw.tile([P, CH, 1], F16)
                        nc.vector.tensor_copy(out=adst16[:], in_=adst_ps[:])
                        ae = pw.tile([P, CH], F16)
                        nc.vector.tensor_scalar(
                            out=ae[:], in0=wv_sb[:, cbase:cbase + CH],
                            scalar1=q2_sb[:, 0:1], scalar2=None, op0=A.mult)
                        nc.vector.tensor_tensor(out=ae[:], in0=ae[:],
                                                in1=adst16[:, :, 0],
                                                op=A.add)
                        nc.vector.tensor_tensor(out=asr, in0=asr,
                                                in1=ae[:].unsqueeze(2),
                                                op=A.add)
                        e2 = pw.tile([P, CH, 1], F16)
                        nc.scalar.activation(out=e2[:], in_=asr,
                                             func=ACT.Exp, scale=0.2)
                        nc.scalar.activation(out=asr, in_=asr, func=ACT.Exp)
                        nc.vector.tensor_tensor(out=asr, in0=asr, in1=e2[:],
                                                op=A.max)
                        gm = G[:, :, 0:HID]
                        nc.vector.tensor_tensor(
                            out=gm, in0=gm,
                            in1=asr.to_broadcast([P, CH, HID]), op=A.mult)
                        for c in range(CH):
                            nc.tensor.matmul(
                                acc[:], lhsT=oh[:, c, :],
                                rhs=G[:, c, 0:HID + 1],
                                start=(hh == 0 and c == 0),
                                stop=(hh == 1 and c == CH - 1))
                    # epilogue: o2 = [relu(acc/denom + b2) | 1]
                    dn = pw.tile([P, 1], F32)
                    nc.vector.tensor_scalar(out=dn[:], in0=acc[:, HID:HID + 1],
                                            scalar1=1e-16, scalar2=None,
                                            op0=A.add)
                    rc = pw.tile([P, 1], F32)
                    nc.vector.reciprocal(rc[:], dn[:])
                    o2 = pw.tile([P, HID + 1], F32)
                    nc.vector.tensor_scalar(out=o2[:, 0:HID],
                                            in0=acc[:, 0:HID],
                                            scalar1=rc[:, 0:1], scalar2=None,
                                            op0=A.mult)
                    nc.vector.tensor_tensor(out=o2[:, 0:HID],
                                            in0=o2[:, 0:HID],
                                            in1=b2_sb[:], op=A.add)
                    nc.vector.tensor_scalar(out=o2[:, 0:HID],
                                            in0=o2[:, 0:HID],
                                            scalar1=0.0, scalar2=None,
                                            op0=A.max)
                    nc.vector.memset(o2[:, HID:HID + 1], 1.0)
                    ohg = poh.tile([P, GPC], F32)
                    nc.vector.tensor_scalar(
                        out=ohg[:], in0=iota_sb[:, 0:GPC],
                        scalar1=gl_sb[:, t:t + 1], scalar2=None,
                        op0=A.is_equal)
                    nc.tensor.matmul(pool_ps[:], lhsT=ohg[:], rhs=o2[:],
                                     start=(t == 0), stop=(t == NT - 1),
                                     skip_group_check=True)

                # ------------- Phase 4: pooled mean + FC --------------------
                fcw_sb = pc.tile([HID, OUT], F32)
                nc.sync.dma_start(fcw_sb[:], fcw_d[:])
                fcb_sb = pc.tile([P, OUT], F32)
                nc.sync.dma_start(fcb_sb[:], fcb_d[:])
                if dbg:
                    plt_sb = pc.tile([GPC, HID + 1], F32)
                    nc.vector.tensor_copy(out=plt_sb[:], in_=pool_ps[:])
                    nc.sync.dma_start(pldbg_d[:], plt_sb[:])
                cnt = pc.tile([GPC, 1], F32)
                nc.vector.tensor_scalar(out=cnt[:],
                                        in0=pool_ps[:, HID:HID + 1],
                                        scalar1=1.0, scalar2=None, op0=A.max)
                rcc = pc.tile([GPC, 1], F32)
                nc.vector.reciprocal(rcc[:], cnt[:])
                pooled = pc.tile([GPC, HID], F32)
                nc.vector.tensor_scalar(out=pooled[:], in0=pool_ps[:, 0:HID],
                                        scalar1=rcc[:, 0:1], scalar2=None,
                                        op0=A.mult)
                pT_ps = ppl.tile([HID, GPC], F32, space="PSUM")
                nc.tensor.transpose(pT_ps[:], pooled[:], ident32[:GPC, :GPC])
                pT = pc.tile([HID, GPC], F32)
                nc.vector.tensor_copy(out=pT[:], in_=pT_ps[:])
                fc_ps = ppl.tile([GPC, OUT], F32, space="PSUM")
                nc.tensor.matmul(fc_ps[:], lhsT=pT[:], rhs=fcw_sb[:],
                                 start=True, stop=True)
                res = pc.tile([GPC, OUT], F32)
                nc.vector.tensor_tensor(out=res[:], in0=fc_ps[:],
                                        in1=fcb_sb[:GPC, :], op=A.add)
                nc.sync.dma_start(out_d[:], res[:])

    nc.compile()
    return nc


# ---------------------------------------------------------------------------
# Entry point.
# ---------------------------------------------------------------------------
def run(inputs, cfg, **run_kwargs):
    in_maps, meta = prepare(inputs, cfg)
    nc = build(meta)
    res = run_bass_kernel_spmd(nc, in_maps, core_ids=list(range(NCORES)),
                               **run_kwargs)
    out = np.concatenate([res.results[c]["out"] for c in range(NCORES)],
                         axis=0)
    return np.asarray(out, np.float32), res


def kernel(**inputs) -> np.ndarray:
    out, _ = run(inputs, FULL_CFG)
    return out
